# revision 15
# baseline (speedup 1.0000x reference)
"""FALCON ObjectSomeValuesFrom forward kernel for Trainium2 (Bass/Tile).

Math: the reference computes
    c_fs[j]   = sigmoid(cw + col_j + b)
    r_fs[i,j] = sigmoid(row_i + col_j + b)
    out[i]    = max_j r_fs[i,j] * c_fs[j]
with col_j = e_j . w_r, row_i = e_i . w_l + rw, cw = c_emb . w_l,
rw = r_emb . w_l.  Both product factors are strictly increasing in col_j,
so the max over j is attained at argmax_j col_j for every i:
    out[i] = sigmoid(a_i + rw + colmax + b) * sigmoid(cw + colmax + b)
with a_i = e_i . w_l and colmax = max_j col_j.  The O(N^2) pairwise block
collapses to two GEMVs over e_all plus an elementwise sigmoid tail.

Implementation: the e-table is transposed on the host to eT [128, 8192]
(k on partitions) and stored in fp8-e4m3 with a power-of-two scale S on
both e and w (products carry S^2, folded into the sigmoid's scale
factor).  Each 128-column block of eT is a natural PE stationary
[K=k, M=128 rows]; rhs = [w_r, w_l] [K=k, N=2] gives out[128 rows, 2] =
both GEMVs per chunk.  64 matmuls fill PSUM [128, 64, 2]; DVE
reduce_max + a GPSIMD partition all-reduce produce colmax, then one
sigmoid over the 64 a-columns and a scalar multiply finish the job.
The e-table DMA is split 56/8 chunks so the matmul drain of the first
slice overlaps the second slice's transfer + completion latency; the
affine map of colmax into the sigmoid bias k1 is folded before the
partition all-reduce (max commutes with it), and the output tail is
fp16 (host upcasts) to shrink the store descriptors.

Every core runs the identical program over the full table (the problem
is latency-dominated; a row-shard would not shorten the critical path,
which is one serial ~1MB DMA plus fixed DMA/semaphore latencies).  The
host gathers row-slice c from core c's output.
"""

import numpy as np

N = 8192        # 8000 named + 192 anon entities
D = 128         # emb dim == contraction == partitions
P = 128
NCORES = 8
RPC = N // NCORES     # rows per core (1024)
NCHUNK = N // P       # 64 chunks of 128 rows
COL_DT = "fp8e3"      # "fp8e4" | "fp8e3" | "fp16" | "bf16"
SCALE = {"fp8e4": 8.0, "fp8e3": 32.0, "fp16": 1.0, "bf16": 1.0}
SPLIT = 56            # chunks in the first e-DMA (0/64 = single DMA)
# Output via prepped SWDGE scatter + trigger_dma. NOTE: functionally
# correct (CoreSim-verified) but TimelineSim's no_exec mode cannot model
# InstIncSwdgeSem's executor-applied sem pre-bumps, so the timing
# simulator deadlocks on any gen_mode==1 prep — keep False.
SCATTER_OUT = False

_CACHE = {}


def _np_dt(col_dt):
    import ml_dtypes
    return {
        "fp8e4": ml_dtypes.float8_e4m3,
        "fp8e3": ml_dtypes.float8_e3m4,
        "fp16": np.float16,
        "bf16": ml_dtypes.bfloat16,
    }[col_dt]


def _build_nc(repeat=1, col_dt=COL_DT):
    import concourse.bass as bass
    import concourse.bacc as bacc
    import concourse.tile as tile
    import concourse.mybir as mybir
    from concourse import bass_isa

    f32 = mybir.dt.float32
    cdt = {
        "fp8e4": mybir.dt.float8e4,
        "fp8e3": mybir.dt.float8e3,
        "fp16": mybir.dt.float16,
        "bf16": mybir.dt.bfloat16,
    }[col_dt]
    inv_s2 = 1.0 / (SCALE[col_dt] * SCALE[col_dt])
    nc = bacc.Bacc("TRN2", target_bir_lowering=False, debug=False)

    # [w_r, w_l] in cols 0:2, then the scaled/transposed/permuted e-table.
    et_d = nc.dram_tensor("et", [P, N + 2], cdt, kind="ExternalInput").ap()
    consts_d = nc.dram_tensor("consts", [P, 2], f32, kind="ExternalInput").ap()
    if SCATTER_OUT:
        idx_d = nc.dram_tensor("idx", [P, NCHUNK // 8], mybir.dt.int16,
                               kind="ExternalInput").ap()
    f16 = mybir.dt.float16
    out_d = nc.dram_tensor("out", [N], f16, kind="ExternalOutput").ap()

    split = SPLIT if 0 < SPLIT < NCHUNK else NCHUNK

    with tile.TileContext(nc) as tc:
        with (
            tc.tile_pool(name="sb", bufs=1) as sb,
            tc.tile_pool(name="ps", bufs=1, space="PSUM") as ps,
        ):
            et = sb.tile([P, N + 2], cdt)
            cut = 2 + split * P
            nc.sync.dma_start(et[:, 0:cut], et_d[:, 0:cut])
            if cut < N + 2:
                nc.sync.dma_start(et[:, cut:], et_d[:, cut:])
            consts_t = sb.tile([P, 2], f32)
            nc.sync.dma_start(consts_t[:], consts_d)
            if SCATTER_OUT:
                idx_t = sb.tile([P, NCHUNK // 8], mybir.dt.int16)
                nc.sync.dma_start(idx_t[:], idx_d)

            # Dependency-free dummy sigmoid: hoists the 1.3us activation
            # table load into the DMA window instead of the critical tail.
            dum = sb.tile([P, 1], f32)
            nc.vector.memset(dum[:], 0.0)
            dum2 = sb.tile([P, 1], f32)
            nc.scalar.activation(
                dum2[:], dum[:], mybir.ActivationFunctionType.Sigmoid
            )

            w2 = et[:, 0:2]
            pst = ps.tile([P, NCHUNK * 2], f32)
            psv = pst[:].rearrange("p (n two) -> p n two", two=2)
            for r in range(repeat):
                for c in range(NCHUNK):
                    nc.tensor.matmul(
                        psv[:, c, :],
                        et[:, 2 + c * P : 2 + (c + 1) * P],
                        w2,
                        start=True,
                        stop=True,
                    )

            # colmax = max over all 8192 col dots (still carrying S^2).
            colm = sb.tile([P, 1], f32)
            nc.vector.reduce_max(colm[:], psv[:, :, 0], axis=mybir.AxisListType.X)
            # Fold the affine map into the per-partition value BEFORE the
            # partition all-reduce (max commutes with x/S^2 + c0), so the
            # Pool output is directly the sigmoid bias k1 — one hop fewer.
            k1p = sb.tile([P, 1], f32)
            nc.vector.tensor_scalar(
                k1p[:], colm[:], inv_s2, consts_t[:, 0:1],
                op0=mybir.AluOpType.mult, op1=mybir.AluOpType.add,
            )
            k1 = sb.tile([P, 1], f32)
            nc.gpsimd.partition_all_reduce(
                k1[:], k1p[:], channels=P, reduce_op=bass_isa.ReduceOp.max
            )
            # k2 = sigmoid(colmax/S^2 + cw + b) = sigmoid(k1 + (cw - rw))
            k2 = sb.tile([P, 1], f32)
            nc.scalar.activation(
                k2[:], k1[:], mybir.ActivationFunctionType.Sigmoid,
                bias=consts_t[:, 1:2], scale=1.0,
            )

            # out[p*64 + c] = sigmoid(a/S^2 + k1) * k2
            so = sb.tile([P, NCHUNK], f16)
            nc.scalar.activation(
                so[:], psv[:, :, 1], mybir.ActivationFunctionType.Sigmoid,
                bias=k1[:, 0:1], scale=inv_s2,
            )
            fo = sb.tile([P, NCHUNK], f16)
            nc.vector.tensor_scalar_mul(fo[:], so[:], k2[:, 0:1])

            if SCATTER_OUT:
                # Descriptors are generated during the DMA window (prep only
                # reads idx_t); the trigger carries the RAW dep on fo.
                dma_sem = nc.alloc_semaphore("out_dma")
                nc.gpsimd.dma_scatter_add(
                    out_d.rearrange("(t e) -> t e", e=NCHUNK),
                    fo[:].rearrange("p (t e) -> p t e", t=1),
                    idx_t[:],
                    P,            # num_idxs: 128 tokens of 64 floats
                    P,
                    NCHUNK,       # elem_size (64 f32 = 256B)
                    prepare_only=True,
                    sem=dma_sem,
                )
                nc.gpsimd.trigger_dma(count=None)
            else:
                outv = out_d.rearrange("(p n) -> p n", p=P)
                nc.sync.dma_start(outv, fo[:])

    nc.compile()
    return nc


def get_nc(repeat=1, col_dt=COL_DT):
    key = ("nc", repeat, col_dt)
    if key not in _CACHE:
        _CACHE[key] = _build_nc(repeat, col_dt)
    return _CACHE[key]


def prepare_in_maps(
    anon_e_emb, e_table, c_table, r_table, fc0_w, fc0_b, c_id, r_id, col_dt=COL_DT
):
    e_all = np.concatenate(
        [np.asarray(e_table, np.float32), np.asarray(anon_e_emb, np.float32)], 0
    )  # [N, D]
    fc0_w = np.asarray(fc0_w, np.float32)
    w_l = fc0_w[0, :D]
    w_r = fc0_w[0, D:]
    b = np.float32(np.asarray(fc0_b, np.float32)[0])
    c_emb = np.asarray(c_table, np.float32)[int(c_id)]
    r_emb = np.asarray(r_table, np.float32)[int(r_id)]
    rw = np.float32(np.dot(r_emb, w_l))
    cw = np.float32(np.dot(c_emb, w_l))

    s = SCALE[col_dt]
    ndt = _np_dt(col_dt)
    # Column permutation: device position 128*c + p (chunk c, out partition
    # p) holds entity 64*p + c, so the PSUM result [p, c] maps to the
    # contiguous "(p n)" DRAM layout for the output store.
    perm = (64 * np.arange(P)[None, :] + np.arange(NCHUNK)[:, None]).reshape(-1)
    aug = np.empty((P, N + 2), np.float32)
    aug[:, 0] = w_r * s
    aug[:, 1] = w_l * s
    aug[:, 2:] = e_all.T[:, perm] * s
    aug = np.ascontiguousarray(aug.astype(ndt))

    consts = np.empty((P, 2), np.float32)
    consts[:, 0] = rw + b
    consts[:, 1] = cw - rw  # k2 bias on top of k1 = colmax/S^2 + rw + b

    in_map = {"et": aug, "consts": consts}
    if SCATTER_OUT:
        # Token i's index lives at idx[i % 16, i // 16] (only the first 16
        # partitions are read; the rest is padding). Identity scatter.
        idx16 = np.arange(P, dtype=np.int16).reshape(P // 16, 16).T  # [16, 8]
        idx = np.tile(idx16, (P // 16, 1))
        in_map["idx"] = np.ascontiguousarray(idx)
    return [in_map] * NCORES


def run(inputs, trace=False, trace_kwargs=None, repeat=1, col_dt=COL_DT):
    from concourse.bass_utils import run_bass_kernel_spmd

    nc = get_nc(repeat, col_dt)
    in_maps = prepare_in_maps(**inputs, col_dt=col_dt)
    res = run_bass_kernel_spmd(
        nc,
        in_maps,
        core_ids=list(range(NCORES)),
        trace=trace,
        **(trace_kwargs or {}),
    )
    out = np.concatenate(
        [res.results[c]["out"][c * RPC : (c + 1) * RPC] for c in range(NCORES)]
    ).astype(np.float32)
    return out, res


def kernel(**inputs) -> np.ndarray:
    out, _ = run(inputs, trace=False)
    return out


# revision 16
# speedup vs baseline: 1.0004x; 1.0004x over previous
"""FALCON ObjectSomeValuesFrom forward kernel for Trainium2 (Bass/Tile).

Math: the reference computes
    c_fs[j]   = sigmoid(cw + col_j + b)
    r_fs[i,j] = sigmoid(row_i + col_j + b)
    out[i]    = max_j r_fs[i,j] * c_fs[j]
with col_j = e_j . w_r, row_i = e_i . w_l + rw, cw = c_emb . w_l,
rw = r_emb . w_l.  Both product factors are strictly increasing in col_j,
so the max over j is attained at argmax_j col_j for every i:
    out[i] = sigmoid(a_i + rw + colmax + b) * sigmoid(cw + colmax + b)
with a_i = e_i . w_l and colmax = max_j col_j.  The O(N^2) pairwise block
collapses to two GEMVs over e_all plus an elementwise sigmoid tail.

Implementation: the e-table is transposed on the host to eT [128, 8192]
(k on partitions) and stored in fp8-e4m3 with a power-of-two scale S on
both e and w (products carry S^2, folded into the sigmoid's scale
factor).  Each 128-column block of eT is a natural PE stationary
[K=k, M=128 rows]; rhs = [w_r, w_l] [K=k, N=2] gives out[128 rows, 2] =
both GEMVs per chunk.  64 matmuls fill PSUM [128, 64, 2]; DVE
reduce_max + a GPSIMD partition all-reduce produce colmax, then one
sigmoid over the 64 a-columns and a scalar multiply finish the job.
The e-table DMA is split 56/8 chunks so the matmul drain of the first
slice overlaps the second slice's transfer + completion latency; the
affine map of colmax into the sigmoid bias k1 is folded before the
partition all-reduce (max commutes with it), and the output tail is
fp16 (host upcasts) to shrink the store descriptors.

Every core runs the identical program over the full table (the problem
is latency-dominated; a row-shard would not shorten the critical path,
which is one serial ~1MB DMA plus fixed DMA/semaphore latencies).  The
host gathers row-slice c from core c's output.
"""

import numpy as np

N = 8192        # 8000 named + 192 anon entities
D = 128         # emb dim == contraction == partitions
P = 128
NCORES = 8
RPC = N // NCORES     # rows per core (1024)
NCHUNK = N // P       # 64 chunks of 128 rows
COL_DT = "fp8e3"      # "fp8e4" | "fp8e3" | "fp16" | "bf16"
SCALE = {"fp8e4": 8.0, "fp8e3": 32.0, "fp16": 1.0, "bf16": 1.0}
SPLIT = 58            # chunks in the first e-DMA (0/64 = single DMA)
# Output via prepped SWDGE scatter + trigger_dma. NOTE: functionally
# correct (CoreSim-verified) but TimelineSim's no_exec mode cannot model
# InstIncSwdgeSem's executor-applied sem pre-bumps, so the timing
# simulator deadlocks on any gen_mode==1 prep — keep False.
SCATTER_OUT = False

_CACHE = {}


def _np_dt(col_dt):
    import ml_dtypes
    return {
        "fp8e4": ml_dtypes.float8_e4m3,
        "fp8e3": ml_dtypes.float8_e3m4,
        "fp16": np.float16,
        "bf16": ml_dtypes.bfloat16,
    }[col_dt]


def _build_nc(repeat=1, col_dt=COL_DT):
    import concourse.bass as bass
    import concourse.bacc as bacc
    import concourse.tile as tile
    import concourse.mybir as mybir
    from concourse import bass_isa

    f32 = mybir.dt.float32
    cdt = {
        "fp8e4": mybir.dt.float8e4,
        "fp8e3": mybir.dt.float8e3,
        "fp16": mybir.dt.float16,
        "bf16": mybir.dt.bfloat16,
    }[col_dt]
    inv_s2 = 1.0 / (SCALE[col_dt] * SCALE[col_dt])
    nc = bacc.Bacc("TRN2", target_bir_lowering=False, debug=False)

    # Cols 0:2 = [w_r, w_l]; 2:N+2 = scaled/transposed/permuted e-table;
    # N+2:N+4 = pad (f32 alignment); N+4:N+12 = the two f32 consts as raw
    # bytes (bitcast view) so one DMA covers every input.
    et_d = nc.dram_tensor("et", [P, N + 12], cdt, kind="ExternalInput").ap()
    if SCATTER_OUT:
        idx_d = nc.dram_tensor("idx", [P, NCHUNK // 8], mybir.dt.int16,
                               kind="ExternalInput").ap()
    f16 = mybir.dt.float16
    out_d = nc.dram_tensor("out", [N], f16, kind="ExternalOutput").ap()

    split = SPLIT if 0 < SPLIT < NCHUNK else NCHUNK

    with tile.TileContext(nc) as tc:
        with (
            tc.tile_pool(name="sb", bufs=1) as sb,
            tc.tile_pool(name="ps", bufs=1, space="PSUM") as ps,
        ):
            et = sb.tile([P, N + 12], cdt)
            cut = 2 + split * P
            nc.sync.dma_start(et[:, 0:cut], et_d[:, 0:cut])
            if cut < N + 12:
                nc.sync.dma_start(et[:, cut:], et_d[:, cut:])
            consts_t = et[:, N + 4 : N + 12].bitcast(f32)
            if SCATTER_OUT:
                idx_t = sb.tile([P, NCHUNK // 8], mybir.dt.int16)
                nc.sync.dma_start(idx_t[:], idx_d)

            # Dependency-free dummy sigmoid: hoists the 1.3us activation
            # table load into the DMA window instead of the critical tail.
            dum = sb.tile([P, 1], f32)
            nc.vector.memset(dum[:], 0.0)
            dum2 = sb.tile([P, 1], f32)
            nc.scalar.activation(
                dum2[:], dum[:], mybir.ActivationFunctionType.Sigmoid
            )

            w2 = et[:, 0:2]
            pst = ps.tile([P, NCHUNK * 2], f32)
            psv = pst[:].rearrange("p (n two) -> p n two", two=2)
            for r in range(repeat):
                for c in range(NCHUNK):
                    nc.tensor.matmul(
                        psv[:, c, :],
                        et[:, 2 + c * P : 2 + (c + 1) * P],
                        w2,
                        start=True,
                        stop=True,
                    )

            # colmax = max over all 8192 col dots (still carrying S^2).
            colm = sb.tile([P, 1], f32)
            nc.vector.reduce_max(colm[:], psv[:, :, 0], axis=mybir.AxisListType.X)
            # Fold the affine map into the per-partition value BEFORE the
            # partition all-reduce (max commutes with x/S^2 + c0), so the
            # Pool output is directly the sigmoid bias k1 — one hop fewer.
            k1p = sb.tile([P, 1], f32)
            nc.vector.tensor_scalar(
                k1p[:], colm[:], inv_s2, consts_t[:, 0:1],
                op0=mybir.AluOpType.mult, op1=mybir.AluOpType.add,
            )
            k1 = sb.tile([P, 1], f32)
            nc.gpsimd.partition_all_reduce(
                k1[:], k1p[:], channels=P, reduce_op=bass_isa.ReduceOp.max
            )
            # k2 = sigmoid(colmax/S^2 + cw + b) = sigmoid(k1 + (cw - rw))
            k2 = sb.tile([P, 1], f32)
            nc.scalar.activation(
                k2[:], k1[:], mybir.ActivationFunctionType.Sigmoid,
                bias=consts_t[:, 1:2], scale=1.0,
            )

            # out[p*64 + c] = sigmoid(a/S^2 + k1) * k2
            so = sb.tile([P, NCHUNK], f16)
            nc.scalar.activation(
                so[:], psv[:, :, 1], mybir.ActivationFunctionType.Sigmoid,
                bias=k1[:, 0:1], scale=inv_s2,
            )
            fo = sb.tile([P, NCHUNK], f16)
            nc.vector.tensor_scalar_mul(fo[:], so[:], k2[:, 0:1])

            if SCATTER_OUT:
                # Descriptors are generated during the DMA window (prep only
                # reads idx_t); the trigger carries the RAW dep on fo.
                dma_sem = nc.alloc_semaphore("out_dma")
                nc.gpsimd.dma_scatter_add(
                    out_d.rearrange("(t e) -> t e", e=NCHUNK),
                    fo[:].rearrange("p (t e) -> p t e", t=1),
                    idx_t[:],
                    P,            # num_idxs: 128 tokens of 64 floats
                    P,
                    NCHUNK,       # elem_size (64 f32 = 256B)
                    prepare_only=True,
                    sem=dma_sem,
                )
                nc.gpsimd.trigger_dma(count=None)
            else:
                outv = out_d.rearrange("(p n) -> p n", p=P)
                nc.sync.dma_start(outv, fo[:])

    nc.compile()
    return nc


def get_nc(repeat=1, col_dt=COL_DT):
    key = ("nc", repeat, col_dt)
    if key not in _CACHE:
        _CACHE[key] = _build_nc(repeat, col_dt)
    return _CACHE[key]


def prepare_in_maps(
    anon_e_emb, e_table, c_table, r_table, fc0_w, fc0_b, c_id, r_id, col_dt=COL_DT
):
    e_all = np.concatenate(
        [np.asarray(e_table, np.float32), np.asarray(anon_e_emb, np.float32)], 0
    )  # [N, D]
    fc0_w = np.asarray(fc0_w, np.float32)
    w_l = fc0_w[0, :D]
    w_r = fc0_w[0, D:]
    b = np.float32(np.asarray(fc0_b, np.float32)[0])
    c_emb = np.asarray(c_table, np.float32)[int(c_id)]
    r_emb = np.asarray(r_table, np.float32)[int(r_id)]
    rw = np.float32(np.dot(r_emb, w_l))
    cw = np.float32(np.dot(c_emb, w_l))

    s = SCALE[col_dt]
    ndt = _np_dt(col_dt)
    # Column permutation: device position 128*c + p (chunk c, out partition
    # p) holds entity 64*p + c, so the PSUM result [p, c] maps to the
    # contiguous "(p n)" DRAM layout for the output store.
    perm = (64 * np.arange(P)[None, :] + np.arange(NCHUNK)[:, None]).reshape(-1)
    consts = np.empty((P, 2), np.float32)
    consts[:, 0] = rw + b
    consts[:, 1] = cw - rw  # k2 bias on top of k1 = colmax/S^2 + rw + b

    augf = np.empty((P, N + 2), np.float32)
    augf[:, 0] = w_r * s
    augf[:, 1] = w_l * s
    augf[:, 2:] = e_all.T[:, perm] * s
    aug = np.zeros((P, N + 12), ndt)
    aug[:, : N + 2] = augf.astype(ndt)
    # consts ride along as raw f32 bytes at 4-byte-aligned col N+4
    aug.view(np.uint8)[:, N + 4 : N + 12] = consts.view(np.uint8)
    aug = np.ascontiguousarray(aug)

    in_map = {"et": aug}
    if SCATTER_OUT:
        # Token i's index lives at idx[i % 16, i // 16] (only the first 16
        # partitions are read; the rest is padding). Identity scatter.
        idx16 = np.arange(P, dtype=np.int16).reshape(P // 16, 16).T  # [16, 8]
        idx = np.tile(idx16, (P // 16, 1))
        in_map["idx"] = np.ascontiguousarray(idx)
    return [in_map] * NCORES


def run(inputs, trace=False, trace_kwargs=None, repeat=1, col_dt=COL_DT):
    from concourse.bass_utils import run_bass_kernel_spmd

    nc = get_nc(repeat, col_dt)
    in_maps = prepare_in_maps(**inputs, col_dt=col_dt)
    res = run_bass_kernel_spmd(
        nc,
        in_maps,
        core_ids=list(range(NCORES)),
        trace=trace,
        **(trace_kwargs or {}),
    )
    out = np.concatenate(
        [res.results[c]["out"][c * RPC : (c + 1) * RPC] for c in range(NCORES)]
    ).astype(np.float32)
    return out, res


def kernel(**inputs) -> np.ndarray:
    out, _ = run(inputs, trace=False)
    return out


# revision 22
# speedup vs baseline: 1.0100x; 1.0096x over previous
"""FALCON ObjectSomeValuesFrom forward kernel for Trainium2 (Bass/Tile).

Math: the reference computes
    c_fs[j]   = sigmoid(cw + col_j + b)
    r_fs[i,j] = sigmoid(row_i + col_j + b)
    out[i]    = max_j r_fs[i,j] * c_fs[j]
with col_j = e_j . w_r, row_i = e_i . w_l + rw, cw = c_emb . w_l,
rw = r_emb . w_l.  Both product factors are strictly increasing in col_j,
so the max over j is attained at argmax_j col_j for every i:
    out[i] = sigmoid(a_i + rw + colmax + b) * sigmoid(cw + colmax + b)
with a_i = e_i . w_l and colmax = max_j col_j.  The O(N^2) pairwise block
collapses to two GEMVs over e_all plus an elementwise sigmoid tail.

Implementation: the e-table is transposed on the host to eT [128, 8192]
(k on partitions) and stored in fp8-e3m4 with a power-of-two scale S on
both e and w (products carry S^2, folded into the sigmoid's scale
factor).  Each 128-column block of eT is a natural PE stationary
[K=k, M=128 rows]; rhs = [w_r, w_l] [K=k, N=2] gives out[128 rows, 2] =
both GEMVs per chunk at ~2 PE cycles each.  64 matmuls fill PSUM
[128, 64, 2]; DVE reduce_max + a GPSIMD partition all-reduce produce
colmax, then one sigmoid over this core's 8 a-columns and a scalar
multiply finish the job.  Latency tricks:
  - one input DMA carries weights, table, and the f32 sigmoid consts
    (bit-cast trailing bytes), split 58/6 chunks so the matmul drain of
    the first slice hides the second slice's transfer + sem latency;
  - the affine map of colmax into the sigmoid bias k1 is folded BEFORE
    the partition all-reduce (max commutes with it) to save a hop;
  - a dependency-free dummy sigmoid hoists the 1.3us activation-table
    load into the DMA window;
  - the output tail is fp16 (host upcasts) and only [128, 8]: each
    core's in_map permutes the eT columns so ITS OWN 1024 rows land in
    chunks 0-7 (colmax is permutation-invariant), giving a contiguous
    1024-element store per core from one shared compiled NEFF.

Every core scans the full table (the problem is latency-dominated and
collectives are far more expensive than the redundant 1MB read; the
critical path is one serial DMA plus fixed DMA/semaphore latencies).
The host concatenates the per-core 1024-row outputs.
"""

import numpy as np

N = 8192        # 8000 named + 192 anon entities
D = 128         # emb dim == contraction == partitions
P = 128
NCORES = 8
RPC = N // NCORES     # rows per core (1024)
NCHUNK = N // P       # 64 chunks of 128 rows
COL_DT = "fp8e3"      # "fp8e4" | "fp8e3" | "fp16" | "bf16"
SCALE = {"fp8e4": 8.0, "fp8e3": 32.0, "fp16": 1.0, "bf16": 1.0}
SPLIT = 58            # chunks in the first e-DMA (0/64 = single DMA)
# Output via prepped SWDGE scatter + trigger_dma. NOTE: functionally
# correct (CoreSim-verified) but TimelineSim's no_exec mode cannot model
# InstIncSwdgeSem's executor-applied sem pre-bumps, so the timing
# simulator deadlocks on any gen_mode==1 prep — keep False.
SCATTER_OUT = False

_CACHE = {}


def _np_dt(col_dt):
    import ml_dtypes
    return {
        "fp8e4": ml_dtypes.float8_e4m3,
        "fp8e3": ml_dtypes.float8_e3m4,
        "fp16": np.float16,
        "bf16": ml_dtypes.bfloat16,
    }[col_dt]


def _build_nc(repeat=1, col_dt=COL_DT):
    import concourse.bass as bass
    import concourse.bacc as bacc
    import concourse.tile as tile
    import concourse.mybir as mybir
    from concourse import bass_isa

    f32 = mybir.dt.float32
    cdt = {
        "fp8e4": mybir.dt.float8e4,
        "fp8e3": mybir.dt.float8e3,
        "fp16": mybir.dt.float16,
        "bf16": mybir.dt.bfloat16,
    }[col_dt]
    inv_s2 = 1.0 / (SCALE[col_dt] * SCALE[col_dt])
    nc = bacc.Bacc("TRN2", target_bir_lowering=False, debug=False)

    # Cols 0:2 = [w_r, w_l]; 2:N+2 = scaled/transposed/permuted e-table;
    # N+2:N+4 = pad (f32 alignment); N+4:N+12 = the two f32 consts as raw
    # bytes (bitcast view) so one DMA covers every input.
    et_d = nc.dram_tensor("et", [P, N + 12], cdt, kind="ExternalInput").ap()
    if SCATTER_OUT:
        idx_d = nc.dram_tensor("idx", [P, NCHUNK // 8], mybir.dt.int16,
                               kind="ExternalInput").ap()
    f16 = mybir.dt.float16
    out_d = nc.dram_tensor("out", [RPC], f16, kind="ExternalOutput").ap()

    split = SPLIT if 0 < SPLIT < NCHUNK else NCHUNK

    with tile.TileContext(nc) as tc:
        with (
            tc.tile_pool(name="sb", bufs=1) as sb,
            tc.tile_pool(name="ps", bufs=1, space="PSUM") as ps,
        ):
            et = sb.tile([P, N + 12], cdt)
            cut = 2 + split * P
            nc.sync.dma_start(et[:, 0:cut], et_d[:, 0:cut])
            if cut < N + 12:
                nc.sync.dma_start(et[:, cut:], et_d[:, cut:])
            consts_t = et[:, N + 4 : N + 12].bitcast(f32)
            if SCATTER_OUT:
                idx_t = sb.tile([P, NCHUNK // 8], mybir.dt.int16)
                nc.sync.dma_start(idx_t[:], idx_d)

            # Dependency-free dummy sigmoid: hoists the 1.3us activation
            # table load into the DMA window instead of the critical tail.
            dum = sb.tile([P, 1], f32)
            nc.vector.memset(dum[:], 0.0)
            dum2 = sb.tile([P, 1], f32)
            nc.scalar.activation(
                dum2[:], dum[:], mybir.ActivationFunctionType.Sigmoid
            )

            w2 = et[:, 0:2]
            pst = ps.tile([P, NCHUNK * 2], f32)
            psv = pst[:].rearrange("p (n two) -> p n two", two=2)
            for r in range(repeat):
                for c in range(NCHUNK):
                    nc.tensor.matmul(
                        psv[:, c, :],
                        et[:, 2 + c * P : 2 + (c + 1) * P],
                        w2,
                        start=True,
                        stop=True,
                    )

            # colmax = max over all 8192 col dots (still carrying S^2).
            colm = sb.tile([P, 1], f32)
            nc.vector.reduce_max(colm[:], psv[:, :, 0], axis=mybir.AxisListType.X)
            # Fold the affine map into the per-partition value BEFORE the
            # partition all-reduce (max commutes with x/S^2 + c0), so the
            # Pool output is directly the sigmoid bias k1 — one hop fewer.
            k1p = sb.tile([P, 1], f32)
            nc.vector.tensor_scalar(
                k1p[:], colm[:], inv_s2, consts_t[:, 0:1],
                op0=mybir.AluOpType.mult, op1=mybir.AluOpType.add,
            )
            k1 = sb.tile([P, 1], f32)
            nc.gpsimd.partition_all_reduce(
                k1[:], k1p[:], channels=P, reduce_op=bass_isa.ReduceOp.max
            )
            # k2 = sigmoid(colmax/S^2 + cw + b) = sigmoid(k1 + (cw - rw))
            k2 = sb.tile([P, 1], f32)
            nc.scalar.activation(
                k2[:], k1[:], mybir.ActivationFunctionType.Sigmoid,
                bias=consts_t[:, 1:2], scale=1.0,
            )

            # This core's own 1024 rows live in chunks 0-7 (the host
            # permutation is per-core), so the sigmoid tail and store touch
            # only [128, 8]: out[8p + n] = sigmoid(a/S^2 + k1) * k2.
            OWN = RPC // P  # 8 own chunks
            so = sb.tile([P, OWN], f16)
            nc.scalar.activation(
                so[:], psv[:, 0:OWN, 1], mybir.ActivationFunctionType.Sigmoid,
                bias=k1[:, 0:1], scale=inv_s2,
            )
            fo = sb.tile([P, OWN], f16)
            nc.vector.tensor_scalar_mul(fo[:], so[:], k2[:, 0:1])

            if SCATTER_OUT:
                # Descriptors are generated during the DMA window (prep only
                # reads idx_t); the trigger carries the RAW dep on fo.
                dma_sem = nc.alloc_semaphore("out_dma")
                nc.gpsimd.dma_scatter_add(
                    out_d.rearrange("(t e) -> t e", e=NCHUNK),
                    fo[:].rearrange("p (t e) -> p t e", t=1),
                    idx_t[:],
                    P,            # num_idxs: 128 tokens of 64 floats
                    P,
                    NCHUNK,       # elem_size (64 f32 = 256B)
                    prepare_only=True,
                    sem=dma_sem,
                )
                nc.gpsimd.trigger_dma(count=None)
            else:
                outv = out_d.rearrange("(p n) -> p n", p=P)
                nc.sync.dma_start(outv, fo[:])

    nc.compile()
    return nc


def get_nc(repeat=1, col_dt=COL_DT):
    key = ("nc", repeat, col_dt)
    if key not in _CACHE:
        _CACHE[key] = _build_nc(repeat, col_dt)
    return _CACHE[key]


def prepare_in_maps(
    anon_e_emb, e_table, c_table, r_table, fc0_w, fc0_b, c_id, r_id, col_dt=COL_DT
):
    e_all = np.concatenate(
        [np.asarray(e_table, np.float32), np.asarray(anon_e_emb, np.float32)], 0
    )  # [N, D]
    fc0_w = np.asarray(fc0_w, np.float32)
    w_l = fc0_w[0, :D]
    w_r = fc0_w[0, D:]
    b = np.float32(np.asarray(fc0_b, np.float32)[0])
    c_emb = np.asarray(c_table, np.float32)[int(c_id)]
    r_emb = np.asarray(r_table, np.float32)[int(r_id)]
    rw = np.float32(np.dot(r_emb, w_l))
    cw = np.float32(np.dot(c_emb, w_l))

    s = SCALE[col_dt]
    ndt = _np_dt(col_dt)
    # Per-core column permutation: colmax is permutation-invariant, so each
    # core's eT columns are ordered so that ITS OWN 1024 rows land in
    # chunks 0-7 at device position 128*n + p = row base + 8*p + n, making
    # the PSUM a-slice [p, n] map to the contiguous out[8p + n] store.
    # The remaining 7168 rows fill chunks 8-63 in arbitrary order.
    own_pos = (128 * (np.arange(RPC) % (RPC // P)) + np.arange(RPC) // (RPC // P))
    consts = np.empty((P, 2), np.float32)
    consts[:, 0] = rw + b
    consts[:, 1] = cw - rw  # k2 bias on top of k1 = colmax/S^2 + rw + b

    eT = np.ascontiguousarray((e_all.T * s).astype(ndt))  # quantize once
    in_maps = []
    for core in range(NCORES):
        perm = np.empty(N, np.int64)
        own = core * RPC + np.arange(RPC)
        perm[own_pos] = own
        rest = np.concatenate([np.arange(0, core * RPC),
                               np.arange((core + 1) * RPC, N)])
        perm[RPC:] = rest
        aug = np.zeros((P, N + 12), ndt)
        aug[:, 0] = (w_r * s).astype(ndt)
        aug[:, 1] = (w_l * s).astype(ndt)
        aug[:, 2 : N + 2] = eT[:, perm]
        # consts ride along as raw f32 bytes at 4-byte-aligned col N+4
        aug.view(np.uint8)[:, N + 4 : N + 12] = consts.view(np.uint8)
        in_maps.append({"et": np.ascontiguousarray(aug)})

    return in_maps


def run(inputs, trace=False, trace_kwargs=None, repeat=1, col_dt=COL_DT):
    from concourse.bass_utils import run_bass_kernel_spmd

    nc = get_nc(repeat, col_dt)
    in_maps = prepare_in_maps(**inputs, col_dt=col_dt)
    res = run_bass_kernel_spmd(
        nc,
        in_maps,
        core_ids=list(range(NCORES)),
        trace=trace,
        **(trace_kwargs or {}),
    )
    out = np.concatenate(
        [res.results[c]["out"] for c in range(NCORES)]
    ).astype(np.float32)
    return out, res


def kernel(**inputs) -> np.ndarray:
    out, _ = run(inputs, trace=False)
    return out


# revision 23
# speedup vs baseline: 1.0108x; 1.0008x over previous
"""FALCON ObjectSomeValuesFrom forward kernel for Trainium2 (Bass/Tile).

Math: the reference computes
    c_fs[j]   = sigmoid(cw + col_j + b)
    r_fs[i,j] = sigmoid(row_i + col_j + b)
    out[i]    = max_j r_fs[i,j] * c_fs[j]
with col_j = e_j . w_r, row_i = e_i . w_l + rw, cw = c_emb . w_l,
rw = r_emb . w_l.  Both product factors are strictly increasing in col_j,
so the max over j is attained at argmax_j col_j for every i:
    out[i] = sigmoid(a_i + rw + colmax + b) * sigmoid(cw + colmax + b)
with a_i = e_i . w_l and colmax = max_j col_j.  The O(N^2) pairwise block
collapses to two GEMVs over e_all plus an elementwise sigmoid tail.

Implementation: the e-table is transposed on the host to eT [128, 8192]
(k on partitions) and stored in fp8-e3m4 with a power-of-two scale S on
both e and w (products carry S^2, folded into the sigmoid's scale
factor).  Each 128-column block of eT is a natural PE stationary
[K=k, M=128 rows]; rhs = [w_r, w_l] [K=k, N=2] gives out[128 rows, 2] =
both GEMVs per chunk at ~2 PE cycles each.  64 matmuls fill PSUM
[128, 64, 2]; DVE reduce_max + a GPSIMD partition all-reduce produce
colmax, then one sigmoid over this core's 8 a-columns and a scalar
multiply finish the job.  Latency tricks:
  - one input DMA carries weights, table, and the f32 sigmoid consts
    (bit-cast trailing bytes), split 58/6 chunks so the matmul drain of
    the first slice hides the second slice's transfer + sem latency;
  - the affine map of colmax into the sigmoid bias k1 is folded BEFORE
    the partition all-reduce (max commutes with it) to save a hop;
  - a dependency-free dummy sigmoid hoists the 1.3us activation-table
    load into the DMA window;
  - the output tail is fp16 (host upcasts) and only [128, 8]: each
    core's in_map permutes the eT columns so ITS OWN 1024 rows land in
    chunks 0-7 (colmax is permutation-invariant), giving a contiguous
    1024-element store per core from one shared compiled NEFF.

Every core scans the full table (the problem is latency-dominated and
collectives are far more expensive than the redundant 1MB read; the
critical path is one serial DMA plus fixed DMA/semaphore latencies).
The host concatenates the per-core 1024-row outputs.
"""

import numpy as np

N = 8192        # 8000 named + 192 anon entities
D = 128         # emb dim == contraction == partitions
P = 128
NCORES = 8
RPC = N // NCORES     # rows per core (1024)
NCHUNK = N // P       # 64 chunks of 128 rows
COL_DT = "fp8e3"      # "fp8e4" | "fp8e3" | "fp16" | "bf16"
SCALE = {"fp8e4": 8.0, "fp8e3": 32.0, "fp16": 1.0, "bf16": 1.0}
SPLITS = (52, 60)     # e-DMA slice boundaries (chunks); cascading
                      # slices let each slice's matmul drain hide inside
                      # the next slice's transfer window. The last slice
                      # must keep >=4 chunks (>=512B/partition) to avoid
                      # the small-descriptor 2x transfer penalty.
# Output via prepped SWDGE scatter + trigger_dma. NOTE: functionally
# correct (CoreSim-verified) but TimelineSim's no_exec mode cannot model
# InstIncSwdgeSem's executor-applied sem pre-bumps, so the timing
# simulator deadlocks on any gen_mode==1 prep — keep False.
SCATTER_OUT = False

_CACHE = {}


def _np_dt(col_dt):
    import ml_dtypes
    return {
        "fp8e4": ml_dtypes.float8_e4m3,
        "fp8e3": ml_dtypes.float8_e3m4,
        "fp16": np.float16,
        "bf16": ml_dtypes.bfloat16,
    }[col_dt]


def _build_nc(repeat=1, col_dt=COL_DT):
    import concourse.bass as bass
    import concourse.bacc as bacc
    import concourse.tile as tile
    import concourse.mybir as mybir
    from concourse import bass_isa

    f32 = mybir.dt.float32
    cdt = {
        "fp8e4": mybir.dt.float8e4,
        "fp8e3": mybir.dt.float8e3,
        "fp16": mybir.dt.float16,
        "bf16": mybir.dt.bfloat16,
    }[col_dt]
    inv_s2 = 1.0 / (SCALE[col_dt] * SCALE[col_dt])
    nc = bacc.Bacc("TRN2", target_bir_lowering=False, debug=False)

    # Cols 0:2 = [w_r, w_l]; 2:N+2 = scaled/transposed/permuted e-table;
    # N+2:N+4 = pad (f32 alignment); N+4:N+12 = the two f32 consts as raw
    # bytes (bitcast view) so one DMA covers every input.
    et_d = nc.dram_tensor("et", [P, N + 12], cdt, kind="ExternalInput").ap()
    if SCATTER_OUT:
        idx_d = nc.dram_tensor("idx", [P, NCHUNK // 8], mybir.dt.int16,
                               kind="ExternalInput").ap()
    f16 = mybir.dt.float16
    out_d = nc.dram_tensor("out", [RPC], f16, kind="ExternalOutput").ap()

    with tile.TileContext(nc) as tc:
        with (
            tc.tile_pool(name="sb", bufs=1) as sb,
            tc.tile_pool(name="ps", bufs=1, space="PSUM") as ps,
        ):
            et = sb.tile([P, N + 12], cdt)
            prev = 0
            for s in SPLITS:
                cut = 2 + s * P
                nc.sync.dma_start(et[:, prev:cut], et_d[:, prev:cut])
                prev = cut
            nc.sync.dma_start(et[:, prev:], et_d[:, prev:])
            consts_t = et[:, N + 4 : N + 12].bitcast(f32)
            if SCATTER_OUT:
                idx_t = sb.tile([P, NCHUNK // 8], mybir.dt.int16)
                nc.sync.dma_start(idx_t[:], idx_d)

            # Dependency-free dummy sigmoid: hoists the 1.3us activation
            # table load into the DMA window instead of the critical tail.
            dum = sb.tile([P, 1], f32)
            nc.vector.memset(dum[:], 0.0)
            dum2 = sb.tile([P, 1], f32)
            nc.scalar.activation(
                dum2[:], dum[:], mybir.ActivationFunctionType.Sigmoid
            )

            w2 = et[:, 0:2]
            pst = ps.tile([P, NCHUNK * 2], f32)
            psv = pst[:].rearrange("p (n two) -> p n two", two=2)
            for r in range(repeat):
                for c in range(NCHUNK):
                    nc.tensor.matmul(
                        psv[:, c, :],
                        et[:, 2 + c * P : 2 + (c + 1) * P],
                        w2,
                        start=True,
                        stop=True,
                    )

            # colmax = max over all 8192 col dots (still carrying S^2).
            colm = sb.tile([P, 1], f32)
            nc.vector.reduce_max(colm[:], psv[:, :, 0], axis=mybir.AxisListType.X)
            # Fold the affine map into the per-partition value BEFORE the
            # partition all-reduce (max commutes with x/S^2 + c0), so the
            # Pool output is directly the sigmoid bias k1 — one hop fewer.
            k1p = sb.tile([P, 1], f32)
            nc.vector.tensor_scalar(
                k1p[:], colm[:], inv_s2, consts_t[:, 0:1],
                op0=mybir.AluOpType.mult, op1=mybir.AluOpType.add,
            )
            k1 = sb.tile([P, 1], f32)
            nc.gpsimd.partition_all_reduce(
                k1[:], k1p[:], channels=P, reduce_op=bass_isa.ReduceOp.max
            )
            # k2 = sigmoid(colmax/S^2 + cw + b) = sigmoid(k1 + (cw - rw))
            k2 = sb.tile([P, 1], f32)
            nc.scalar.activation(
                k2[:], k1[:], mybir.ActivationFunctionType.Sigmoid,
                bias=consts_t[:, 1:2], scale=1.0,
            )

            # This core's own 1024 rows live in chunks 0-7 (the host
            # permutation is per-core), so the sigmoid tail and store touch
            # only [128, 8]: out[8p + n] = sigmoid(a/S^2 + k1) * k2.
            OWN = RPC // P  # 8 own chunks
            so = sb.tile([P, OWN], f16)
            nc.scalar.activation(
                so[:], psv[:, 0:OWN, 1], mybir.ActivationFunctionType.Sigmoid,
                bias=k1[:, 0:1], scale=inv_s2,
            )
            fo = sb.tile([P, OWN], f16)
            nc.vector.tensor_scalar_mul(fo[:], so[:], k2[:, 0:1])

            if SCATTER_OUT:
                # Descriptors are generated during the DMA window (prep only
                # reads idx_t); the trigger carries the RAW dep on fo.
                dma_sem = nc.alloc_semaphore("out_dma")
                nc.gpsimd.dma_scatter_add(
                    out_d.rearrange("(t e) -> t e", e=NCHUNK),
                    fo[:].rearrange("p (t e) -> p t e", t=1),
                    idx_t[:],
                    P,            # num_idxs: 128 tokens of 64 floats
                    P,
                    NCHUNK,       # elem_size (64 f32 = 256B)
                    prepare_only=True,
                    sem=dma_sem,
                )
                nc.gpsimd.trigger_dma(count=None)
            else:
                outv = out_d.rearrange("(p n) -> p n", p=P)
                nc.sync.dma_start(outv, fo[:])

    nc.compile()
    return nc


def get_nc(repeat=1, col_dt=COL_DT):
    key = ("nc", repeat, col_dt)
    if key not in _CACHE:
        _CACHE[key] = _build_nc(repeat, col_dt)
    return _CACHE[key]


def prepare_in_maps(
    anon_e_emb, e_table, c_table, r_table, fc0_w, fc0_b, c_id, r_id, col_dt=COL_DT
):
    e_all = np.concatenate(
        [np.asarray(e_table, np.float32), np.asarray(anon_e_emb, np.float32)], 0
    )  # [N, D]
    fc0_w = np.asarray(fc0_w, np.float32)
    w_l = fc0_w[0, :D]
    w_r = fc0_w[0, D:]
    b = np.float32(np.asarray(fc0_b, np.float32)[0])
    c_emb = np.asarray(c_table, np.float32)[int(c_id)]
    r_emb = np.asarray(r_table, np.float32)[int(r_id)]
    rw = np.float32(np.dot(r_emb, w_l))
    cw = np.float32(np.dot(c_emb, w_l))

    s = SCALE[col_dt]
    ndt = _np_dt(col_dt)
    # Per-core column permutation: colmax is permutation-invariant, so each
    # core's eT columns are ordered so that ITS OWN 1024 rows land in
    # chunks 0-7 at device position 128*n + p = row base + 8*p + n, making
    # the PSUM a-slice [p, n] map to the contiguous out[8p + n] store.
    # The remaining 7168 rows fill chunks 8-63 in arbitrary order.
    own_pos = (128 * (np.arange(RPC) % (RPC // P)) + np.arange(RPC) // (RPC // P))
    consts = np.empty((P, 2), np.float32)
    consts[:, 0] = rw + b
    consts[:, 1] = cw - rw  # k2 bias on top of k1 = colmax/S^2 + rw + b

    eT = np.ascontiguousarray((e_all.T * s).astype(ndt))  # quantize once
    in_maps = []
    for core in range(NCORES):
        perm = np.empty(N, np.int64)
        own = core * RPC + np.arange(RPC)
        perm[own_pos] = own
        rest = np.concatenate([np.arange(0, core * RPC),
                               np.arange((core + 1) * RPC, N)])
        perm[RPC:] = rest
        aug = np.zeros((P, N + 12), ndt)
        aug[:, 0] = (w_r * s).astype(ndt)
        aug[:, 1] = (w_l * s).astype(ndt)
        aug[:, 2 : N + 2] = eT[:, perm]
        # consts ride along as raw f32 bytes at 4-byte-aligned col N+4
        aug.view(np.uint8)[:, N + 4 : N + 12] = consts.view(np.uint8)
        in_maps.append({"et": np.ascontiguousarray(aug)})

    return in_maps


def run(inputs, trace=False, trace_kwargs=None, repeat=1, col_dt=COL_DT):
    from concourse.bass_utils import run_bass_kernel_spmd

    nc = get_nc(repeat, col_dt)
    in_maps = prepare_in_maps(**inputs, col_dt=col_dt)
    res = run_bass_kernel_spmd(
        nc,
        in_maps,
        core_ids=list(range(NCORES)),
        trace=trace,
        **(trace_kwargs or {}),
    )
    out = np.concatenate(
        [res.results[c]["out"] for c in range(NCORES)]
    ).astype(np.float32)
    return out, res


def kernel(**inputs) -> np.ndarray:
    out, _ = run(inputs, trace=False)
    return out
